# revision 1
# baseline (speedup 1.0000x reference)
"""Bass/Trainium2 kernel for nn_GRU_52355651338739.

2-layer GRU, B=32, T=2048, D=256, H=512, followed by sum over hidden dim,
transpose and |s*k|.  Data-parallel over batch across 8 NeuronCores
(B_local=4 per core, GRU weights replicated).

Device program per core (4 phases):
  1. xg0 = x @ W_ih0.T + bias  (hoisted input GEMM, all timesteps)
  2. layer-0 scan over T steps (weights stationary on PE, h streamed)
  3. xg1 = h0 @ W_ih1.T + bias
  4. layer-1 scan + per-chunk column-sum matmul (ones vector) + |k*s| tail

Layouts keep hidden units on SBUF partitions so elementwise gate math uses
all 128 lanes, and h_new is produced directly in next step's matmul-rhs
layout (no transposes inside the scan).
"""

import os
import sys

import numpy as np

sys.path.insert(0, "/opt/trn_rl_repo")

import concourse.bacc as bacc  # noqa: E402
import concourse.bass as bass  # noqa: E402
import concourse.mybir as mybir  # noqa: E402
from concourse.tile import TileContext  # noqa: E402
from concourse.bass_utils import run_bass_kernel_spmd  # noqa: E402

import ml_dtypes  # noqa: E402

BF16_NP = ml_dtypes.bfloat16

# Problem constants
B, T, D, H = 32, 2048, 256, 512
NCORES = 8
BL = B // NCORES          # 4 sequences per core
G3 = 3 * H                # 1536 gate rows
MT = G3 // 128            # 12 M-tiles
KT0 = D // 128            # 2 K-tiles for layer-0 input GEMM
KT = H // 128             # 4 K-tiles for hidden contraction
U = 8                     # scan steps per chunk
NCH = T // U              # 256 chunks
SB = KT * BL              # 16 = h columns per step
NGC = (T * BL) // 512     # 16 GEMM n-chunks of 512 (t,b) columns
SCPG = 512 // (U * BL)    # 16 scan-chunks per GEMM n-chunk

F32 = mybir.dt.float32
BF16 = mybir.dt.bfloat16
AF = mybir.ActivationFunctionType

_CACHED_NC = None


def _gemm_phase(nc, w_sb, bias_sb, xg_dram_r, kt_in, load_rhs):
    """xg[mt, (t,b)] = sum_k W[k,mt].T @ rhs[k] + bias[mt].

    w_sb: SBUF (128, kt_in*G3) bf16 weight tiles.
    xg_dram_r: DRAM AP view (128, NCH, MT, U*BL) to write.
    load_rhs(c, k, dst): DMA (128, 512) bf16 rhs K-tile for n-chunk c.
    """
    with (
        nc.tc.tile_pool(name="gemm_rhs", bufs=3) as rhs_pool,
        nc.tc.tile_pool(name="gemm_ps", bufs=4, space="PSUM") as ps_pool,
        nc.tc.tile_pool(name="gemm_out", bufs=3) as out_pool,
    ):
        for c in range(NGC):
            rhs_tiles = []
            for k in range(kt_in):
                rt = rhs_pool.tile([128, 512], BF16, tag=f"rhs{k}")
                load_rhs(c, k, rt)
                rhs_tiles.append(rt)
            for mt in range(MT):
                ps = ps_pool.tile([128, 512], F32, tag="ps")
                for k in range(kt_in):
                    nc.tensor.matmul(
                        ps[:],
                        w_sb[:, k, mt * 128:(mt + 1) * 128],
                        rhs_tiles[k][:],
                        start=(k == 0),
                        stop=(k == kt_in - 1),
                    )
                xs = out_pool.tile([128, 512], F32, tag="xs")
                nc.vector.tensor_scalar_add(xs[:], ps[:], bias_sb[:, mt: mt + 1])
                nc.sync.dma_start(
                    xg_dram_r[:, c * SCPG:(c + 1) * SCPG, mt, :], xs[:]
                )


def _scan_phase(nc, w_sb, xg_dram_r, bhn_sb, layer, h0_dram=None,
                ones_sb=None, s_dram=None):
    """Sequential GRU scan. layer 0 stores h chunks; layer 1 accumulates s."""
    tc = nc.tc
    with (
        tc.tile_pool(name=f"scan_state{layer}", bufs=1) as state_pool,
        tc.tile_pool(name=f"scan_ew{layer}", bufs=3) as ew_pool,
        tc.tile_pool(name=f"scan_ps{layer}", bufs=2, space="PSUM") as ps_pool,
    ):
        # persistent chunk buffers; slot 0 = carry from previous chunk
        hbf = state_pool.tile([128, U + 1, KT, BL], BF16, tag="hbf")
        hf32 = state_pool.tile([128, U + 1, KT, BL], F32, tag="hf32")
        xg_sb = state_pool.tile([128, MT, U, BL], F32, tag="xg")
        nc.vector.memset(hbf[:, 0], 0)
        nc.vector.memset(hf32[:, 0], 0)

        with tc.For_i(0, NCH, 1) as ci:
            nc.sync.dma_start(xg_sb.rearrange("p m j b -> p m (j b)"), xg_dram_r[:, ci])
            for j in range(U):
                pr = ps_pool.tile([128, KT, BL], F32, tag="pr")
                pz = ps_pool.tile([128, KT, BL], F32, tag="pz")
                pn = ps_pool.tile([128, KT, BL], F32, tag="pn")
                psums = (pr, pz, pn)
                for k in range(KT):
                    rhs = hbf[:, j, k, :]
                    for g in range(3):
                        for m in range(KT):
                            nc.tensor.matmul(
                                psums[g][:, m, :],
                                w_sb[:, k, g * H + m * 128: g * H + (m + 1) * 128],
                                rhs,
                                start=(k == 0 and m == 0),
                                stop=(k == KT - 1 and m == KT - 1),
                                skip_group_check=True,
                            )
                xr = xg_sb[:, 0:KT, j, :]
                xz = xg_sb[:, KT:2 * KT, j, :]
                xn = xg_sb[:, 2 * KT:3 * KT, j, :]
                tr = ew_pool.tile([128, KT, BL], F32, tag="tr")
                nc.vector.tensor_add(tr[:], pr[:], xr)
                rg = ew_pool.tile([128, KT, BL], F32, tag="rg")
                nc.scalar.activation(rg[:], tr[:], AF.Sigmoid)
                tz = ew_pool.tile([128, KT, BL], F32, tag="tz")
                nc.vector.tensor_add(tz[:], pz[:], xz)
                zg = ew_pool.tile([128, KT, BL], F32, tag="zg")
                nc.scalar.activation(zg[:], tz[:], AF.Sigmoid)
                tn = ew_pool.tile([128, KT, BL], F32, tag="tn")
                nc.vector.tensor_add(tn[:], pn[:], bhn_sb[:])
                nc.vector.tensor_mul(tn[:], tn[:], rg[:])
                nc.vector.tensor_add(tn[:], tn[:], xn)
                ng = ew_pool.tile([128, KT, BL], F32, tag="ng")
                nc.scalar.activation(ng[:], tn[:], AF.Tanh)
                # h_new = n + z*(h - n)
                td = ew_pool.tile([128, KT, BL], F32, tag="td")
                nc.vector.tensor_sub(td[:], hf32[:, j], ng[:])
                nc.vector.tensor_mul(td[:], td[:], zg[:])
                nc.vector.tensor_add(hf32[:, j + 1], ng[:], td[:])
                nc.vector.tensor_copy(hbf[:, j + 1], hf32[:, j + 1])
            # carry
            nc.vector.tensor_copy(hbf[:, 0], hbf[:, U])
            nc.vector.tensor_copy(hf32[:, 0], hf32[:, U])
            if layer == 0:
                for k in range(KT):
                    nc.sync.dma_start(h0_dram[k, ci], hbf[:, 1:U + 1, k, :])
            else:
                sps = ps_pool.tile([1, U * SB], F32, tag="sps")
                nc.tensor.matmul(
                    sps[:],
                    ones_sb[:],
                    hf32[:, 1:U + 1],
                    start=True,
                    stop=True,
                )
                ssb = ew_pool.tile([1, U * SB], F32, tag="ssb")
                nc.vector.tensor_copy(ssb[:], sps[:])
                nc.sync.dma_start(s_dram[ci], ssb[:])


def _build_nc(dbg=False):
    nc = bacc.Bacc(None, target_bir_lowering=False, debug=True)

    xT = nc.dram_tensor("xT", [D, T * BL], BF16, kind="ExternalInput")
    whh0 = nc.dram_tensor("whh0", [KT, 128, G3], BF16, kind="ExternalInput")
    wih0 = nc.dram_tensor("wih0", [KT0, 128, G3], BF16, kind="ExternalInput")
    whh1 = nc.dram_tensor("whh1", [KT, 128, G3], BF16, kind="ExternalInput")
    wih1 = nc.dram_tensor("wih1", [KT, 128, G3], BF16, kind="ExternalInput")
    bias0 = nc.dram_tensor("bias0", [128, MT], F32, kind="ExternalInput")
    bias1 = nc.dram_tensor("bias1", [128, MT], F32, kind="ExternalInput")
    bhn0 = nc.dram_tensor("bhn0", [128, KT, BL], F32, kind="ExternalInput")
    bhn1 = nc.dram_tensor("bhn1", [128, KT, BL], F32, kind="ExternalInput")
    onesv = nc.dram_tensor("onesv", [128, 1], F32, kind="ExternalInput")
    krep = nc.dram_tensor("krep", [128, 1], F32, kind="ExternalInput")
    out = nc.dram_tensor("out", [BL, T], F32, kind="ExternalOutput")

    with TileContext(nc) as tc:
        nc.tc = tc
        with (
            tc.tile_pool(name="wpool", bufs=1) as wpool,
            tc.tile_pool(name="dram", bufs=1, space="DRAM") as dpool,
            tc.tile_pool(name="tail", bufs=1) as tail_pool,
        ):
            # resident weights / constants
            whh0_sb = wpool.tile([128, KT, G3], BF16, tag="whh0")
            nc.sync.dma_start(whh0_sb[:], whh0.rearrange("k p c -> p k c"))
            wih0_sb = wpool.tile([128, KT0, G3], BF16, tag="wih0")
            nc.sync.dma_start(wih0_sb[:], wih0.rearrange("k p c -> p k c"))
            whh1_sb = wpool.tile([128, KT, G3], BF16, tag="whh1")
            nc.sync.dma_start(whh1_sb[:], whh1.rearrange("k p c -> p k c"))
            wih1_sb = wpool.tile([128, KT, G3], BF16, tag="wih1")
            nc.sync.dma_start(wih1_sb[:], wih1.rearrange("k p c -> p k c"))
            bias0_sb = wpool.tile([128, MT], F32, tag="bias0")
            nc.sync.dma_start(bias0_sb[:], bias0[:])
            bias1_sb = wpool.tile([128, MT], F32, tag="bias1")
            nc.sync.dma_start(bias1_sb[:], bias1[:])
            bhn0_sb = wpool.tile([128, KT, BL], F32, tag="bhn0")
            nc.sync.dma_start(bhn0_sb[:], bhn0[:])
            bhn1_sb = wpool.tile([128, KT, BL], F32, tag="bhn1")
            nc.sync.dma_start(bhn1_sb[:], bhn1[:])
            ones_sb = wpool.tile([128, 1], F32, tag="ones")
            nc.sync.dma_start(ones_sb[:], onesv[:])
            krep_sb = wpool.tile([128, 1], F32, tag="krep")
            nc.sync.dma_start(krep_sb[:], krep[:])

            # scratch DRAM
            if dbg:
                xg0_dram = nc.dram_tensor("xg0d", [NCH, MT, 128, U * BL], F32, kind="ExternalOutput")
                xg1_dram = nc.dram_tensor("xg1d", [NCH, MT, 128, U * BL], F32, kind="ExternalOutput")
                h0_dram = nc.dram_tensor("h0d", [KT, NCH, 128, U * BL], BF16, kind="ExternalOutput")
                s_dram = nc.dram_tensor("sd", [NCH, 1, U * SB], F32, kind="ExternalOutput")
            else:
                xg0_dram = dpool.tile([NCH, MT, 128, U * BL], F32, tag="xg0")
                xg1_dram = dpool.tile([NCH, MT, 128, U * BL], F32, tag="xg1")
                h0_dram = dpool.tile([KT, NCH, 128, U * BL], BF16, tag="h0")
                s_dram = dpool.tile([NCH, 1, U * SB], F32, tag="s")

            xg0_r = xg0_dram.rearrange("c m p q -> p c m q")
            xg1_r = xg1_dram.rearrange("c m p q -> p c m q")

            # phase 1: xg0 GEMM
            def load_rhs0(c, k, dst):
                nc.sync.dma_start(
                    dst[:], xT[k * 128:(k + 1) * 128, c * 512:(c + 1) * 512]
                )

            with nc.named_scope("gemm0"):
                _gemm_phase(nc, wih0_sb, bias0_sb, xg0_r, KT0, load_rhs0)

            # phase 2: layer-0 scan
            with nc.named_scope("scan0"):
                _scan_phase(nc, whh0_sb, xg0_r, bhn0_sb, 0, h0_dram=h0_dram)

            # phase 3: xg1 GEMM
            h0_rv = h0_dram.rearrange("k c p q -> p k c q")

            def load_rhs1(c, k, dst):
                nc.sync.dma_start(
                    dst[:], h0_rv[:, k, c * SCPG:(c + 1) * SCPG, :]
                )

            with nc.named_scope("gemm1"):
                _gemm_phase(nc, wih1_sb, bias1_sb, xg1_r, KT, load_rhs1)

            # phase 4: layer-1 scan with running sum
            with nc.named_scope("scan1"):
                _scan_phase(nc, whh1_sb, xg1_r, bhn1_sb, 1, ones_sb=ones_sb, s_dram=s_dram)

            # tail: s (NCH, U*SB) -> out (BL, T);  t = (ti*128+p)*U + j
            s_flat = s_dram.rearrange("c o q -> (c o) q")
            for ti in range(NCH // 128):
                st = tail_pool.tile([128, U, KT, BL], F32, tag="st")
                nc.sync.dma_start(st.rearrange("p j k b -> p (j k b)"), s_flat[ti * 128:(ti + 1) * 128, :])
                acc = tail_pool.tile([128, U, BL], F32, tag="acc")
                nc.vector.tensor_add(acc[:], st[:, :, 0, :], st[:, :, 1, :])
                nc.vector.tensor_add(acc[:], acc[:], st[:, :, 2, :])
                nc.vector.tensor_add(acc[:], acc[:], st[:, :, 3, :])
                oabs = tail_pool.tile([128, U, BL], F32, tag="oabs")
                nc.scalar.activation(oabs[:], acc[:], AF.Abs,
                                     scale=krep_sb[:, 0:1])
                for b in range(BL):
                    nc.sync.dma_start(
                        out[b, ti * 128 * U:(ti + 1) * 128 * U]
                        .rearrange("(p j) -> p j", p=128),
                        oabs[:, :, b],
                    )
        nc.tc = None
    nc.finalize()
    return nc


def _get_nc():
    global _CACHED_NC
    if _CACHED_NC is None:
        _CACHED_NC = _build_nc()
    return _CACHED_NC


def _prep_inputs(x, W_ih0, W_hh0, b_ih0, b_hh0, W_ih1, W_hh1, b_ih1, b_hh1, k):
    def wtile(w, kt):  # (3H, Hin) -> [kt, 128, 3H] bf16 (transposed tiles)
        return np.ascontiguousarray(
            w.T.reshape(kt, 128, G3).astype(BF16_NP))

    whh0 = wtile(W_hh0, KT)
    wih0 = wtile(W_ih0, KT0)
    whh1 = wtile(W_hh1, KT)
    wih1 = wtile(W_ih1, KT)

    def bias_comb(b_ih, b_hh):  # (128, MT) f32; n-gate keeps only b_ih
        b = b_ih.astype(np.float64).copy()
        b[:2 * H] += b_hh[:2 * H].astype(np.float64)
        return np.ascontiguousarray(
            b.reshape(MT, 128).T.astype(np.float32))

    bias0 = bias_comb(b_ih0, b_hh0)
    bias1 = bias_comb(b_ih1, b_hh1)

    def bhn(b_hh):  # (128, KT, BL) replicated n-gate hidden bias
        v = b_hh[2 * H:].reshape(KT, 128).T.astype(np.float32)
        return np.ascontiguousarray(
            np.repeat(v[:, :, None], BL, axis=2))

    bhn0 = bhn(b_hh0)
    bhn1 = bhn(b_hh1)
    onesv = np.ones((128, 1), np.float32)
    krep = np.full((128, 1), abs(float(k[0])), np.float32)

    shared = dict(whh0=whh0, wih0=wih0, whh1=whh1, wih1=wih1,
                  bias0=bias0, bias1=bias1, bhn0=bhn0, bhn1=bhn1,
                  onesv=onesv, krep=krep)
    in_maps = []
    for c in range(NCORES):
        xs = x[c * BL:(c + 1) * BL]            # (BL, T, D)
        xT = np.ascontiguousarray(
            xs.transpose(2, 1, 0).reshape(D, T * BL).astype(BF16_NP))
        in_maps.append(dict(xT=xT, **shared))
    return in_maps


def kernel(**inputs):
    nc = _get_nc()
    in_maps = _prep_inputs(**inputs)
    trace = bool(int(os.environ.get("GRU_TRACE", "0")))
    res = run_bass_kernel_spmd(nc, in_maps, list(range(NCORES)), trace=trace)
    if trace and res.exec_time_ns is not None:
        print(f"HW exec time: {res.exec_time_ns} ns")
    out = np.concatenate([res.results[c]["out"] for c in range(NCORES)], axis=0)
    return np.ascontiguousarray(out[..., None].astype(np.float32))

